# revision 4
# baseline (speedup 1.0000x reference)
"""BinaryLinear v14: v9 + software-pipelined repeat loop.

Same single-iteration dataflow as v9 (top-byte inputs, DVE bitwise sign to
fp8 DoubleRow tiles, staircase loads, 1024-MM fp8 DoubleRow stream). The
repeat loop is 2x-unrolled with ping-pong corner tiles so iteration i+1's
first (m,n) corner loads and signs overlap iteration i's matmul tail:
all input loads ride the ACT HWDGE ring, all y stores the SP ring, and the
bias load is hoisted out of the loop, so neither ring head-of-line blocks
the next iteration's prefetch.
"""

import os
import sys
import time

sys.path.insert(0, "/opt/trn_rl_repo")

if "jax" not in sys.modules and os.environ.get("JAX_PLATFORMS") in ("cpu",):
    del os.environ["JAX_PLATFORMS"]

import numpy as np
import ml_dtypes

import concourse.bass as bass
import concourse.mybir as mybir
import concourse.tile as tile

N_TOK = 8192
IN_F = 4096
OUT_F = 4096
R = 4
C = 2
N_CORES = 8
TOK_SH = N_TOK // R  # 2048
OUT_SH = OUT_F // C  # 2048
P = 128
KS = IN_F // P  # 32 k-subtiles
O_CHUNK = 512
M_CHUNK = 512
SB = 4  # k-slices per load/sign batch

f32 = mybir.dt.float32
bf16 = mybir.dt.bfloat16
fp8 = mybir.dt.float8e4
u32 = mybir.dt.uint32


def _split_multi_waits(nc, limit=1):
    """walrus allows one sync-wait per instruction; move extras onto
    preceding NoOps (engines are in-order, so semantics are unchanged)."""
    for f in nc.m.functions:
        for bb in f.blocks:
            new = []
            for inst in bb.instructions:
                si = inst.sync_info
                if si is not None and len(si.on_wait) > limit:
                    waits = list(si.on_wait)
                    extra, keep = waits[:-limit], waits[-limit:]
                    for j, w in enumerate(extra):
                        new.append(
                            mybir.InstNoOp(
                                name=f"{inst.name}-w{j}",
                                engine=inst.engine,
                                sync_info=mybir.SyncInfo(on_wait=[w], on_update=[]),
                            )
                        )
                    inst.sync_info = mybir.SyncInfo(
                        on_wait=keep, on_update=list(si.on_update)
                    )
                new.append(inst)
            bb.instructions = new


def build_nc(repeat=1):
    nc = bass.Bass()
    xT = nc.declare_dram_parameter("xT", [IN_F, TOK_SH // 4], u32, isOutput=False)
    wT = nc.declare_dram_parameter("wT", [IN_F, OUT_SH // 4], u32, isOutput=False)
    b = nc.declare_dram_parameter("b", [P, OUT_SH], f32, isOutput=False)
    y = nc.declare_dram_parameter("y", [TOK_SH, OUT_SH], bf16, isOutput=True)

    nparity = 1 if repeat == 1 else 2

    with tile.TileContext(nc) as tc:
        with (
            tc.tile_pool(name="const", bufs=1) as const,
            tc.tile_pool(name="stage", bufs=10) as stage,
            tc.tile_pool(name="big", bufs=1) as big,
            tc.tile_pool(name="psum", bufs=8, space="PSUM") as psum_pool,
            tc.tile_pool(name="outp", bufs=6) as out_pool,
        ):
            bias_bc = const.tile([P, OUT_SH], f32)

            # Main tiles hold m 512:2048 / n 512:2048; the first (m,n) corner
            # lives in per-parity ping-pong tiles so iteration i+1's corner
            # prefetch never write-after-read conflicts with iteration i.
            xbt = big.tile([P, KS, TOK_SH - M_CHUNK], fp8)
            wbt = big.tile([P, KS, OUT_SH - O_CHUNK], fp8)
            cxs = [
                big.tile([P, KS, M_CHUNK], fp8, name=f"cx{i}") for i in range(nparity)
            ]
            cws = [
                big.tile([P, KS, O_CHUNK], fp8, name=f"cw{i}") for i in range(nparity)
            ]

            def load_batch(src, dst, dram_c0, dst_c0, sb0, nb=SB):
                """Load k-slices [sb0, sb0+nb) of byte-cols [dram_c0,
                dram_c0+512) of src; DVE bitwise sign -> fp8 into dst."""
                cu = dram_c0 // 4
                st = stage.tile([P, SB, M_CHUNK // 4], u32, tag="st")
                nc.scalar.dma_start(
                    out=st[:, 0:nb, :],
                    in_=src[sb0 * P : (sb0 + nb) * P, cu : cu + M_CHUNK // 4].rearrange(
                        "(s p) c -> p s c", s=nb
                    ),
                )
                nc.vector.tensor_scalar(
                    dst[:, sb0 : sb0 + nb, dst_c0 : dst_c0 + M_CHUNK].bitcast(u32),
                    st[:, 0:nb, :],
                    0x80808080,
                    0x38383838,
                    mybir.AluOpType.bitwise_and,
                    mybir.AluOpType.bitwise_or,
                )

            def load_chunk(src, dst, dram_c0, dst_c0):
                for sb0 in range(0, KS, SB):
                    load_batch(src, dst, dram_c0, dst_c0, sb0)

            def load_corner(par):
                for sb0 in range(0, KS, 2):
                    load_batch(xT, cxs[par], 0, 0, sb0, nb=2)
                    load_batch(wT, cws[par], 0, 0, sb0, nb=2)

            def emit_mm(mt, oc, par):
                if mt < 4:
                    lsrc, lc0 = cxs[par], mt * P
                else:
                    lsrc, lc0 = xbt, (mt - 4) * P
                if oc == 0:
                    rsrc, rc0 = cws[par], 0
                else:
                    rsrc, rc0 = wbt, (oc - 1) * O_CHUNK
                ps = psum_pool.tile([P, O_CHUNK], f32, tag="ps")
                for kp in range(KS // 2):
                    nc.tensor.matmul(
                        ps[:],
                        lhsT=lsrc[:, 2 * kp : 2 * kp + 2, lc0 : lc0 + P],
                        rhs=rsrc[:, 2 * kp : 2 * kp + 2, rc0 : rc0 + O_CHUNK],
                        start=(kp == 0),
                        stop=(kp == KS // 2 - 1),
                        perf_mode=mybir.MatmulPerfMode.DoubleRow,
                    )
                out_sb = out_pool.tile([P, O_CHUNK], bf16, tag="out_sb")
                nc.vector.tensor_add(
                    out=out_sb[:],
                    in0=ps[:],
                    in1=bias_bc[:, oc * O_CHUNK : (oc + 1) * O_CHUNK],
                )
                row0 = mt * P
                nc.sync.dma_start(
                    out=y[row0 : row0 + P, oc * O_CHUNK : (oc + 1) * O_CHUNK],
                    in_=out_sb[:],
                )

            def body(par):
                load_corner(par)
                for mt in range(4):
                    emit_mm(mt, 0, par)
                load_chunk(xT, xbt, 512, 0)
                for mt in range(4, 8):
                    emit_mm(mt, 0, par)
                load_chunk(wT, wbt, 512, 0)
                for mt in range(8):
                    emit_mm(mt, 1, par)
                load_chunk(xT, xbt, 1024, 512)
                for mt in range(8, 12):
                    emit_mm(mt, 0, par)
                    emit_mm(mt, 1, par)
                load_chunk(xT, xbt, 1536, 1024)
                for mt in range(12, 16):
                    emit_mm(mt, 0, par)
                    emit_mm(mt, 1, par)
                load_chunk(wT, wbt, 1024, 512)
                for mt in range(16):
                    emit_mm(mt, 2, par)
                load_chunk(wT, wbt, 1536, 1024)
                for mt in range(16):
                    emit_mm(mt, 3, par)

            nc.sync.dma_start(out=bias_bc[:], in_=b[:])
            if repeat == 1:
                body(0)
            else:
                with tc.For_i(0, repeat // 2, 1):
                    body(0)
                    body(1)
                if repeat % 2:
                    body(0)

    _split_multi_waits(nc)
    return nc


_cached_nc = None


def _get_nc():
    global _cached_nc
    if _cached_nc is None:
        _cached_nc = build_nc()
    return _cached_nc


def _in_maps(x, weight, bias):
    # Top byte of each little-endian f32 = sign bit + 7 high exponent bits.
    # Pure byte movement; sign() itself runs on device (DVE bitwise).
    xv = x.view(np.uint8)[:, 3::4]  # [N_TOK, IN_F]
    wv = weight.view(np.uint8)[:, 3::4]  # [OUT_F, IN_F]
    xts = [
        np.ascontiguousarray(xv[r * TOK_SH : (r + 1) * TOK_SH].T).view(np.uint32)
        for r in range(R)
    ]
    wts = [
        np.ascontiguousarray(wv[h * OUT_SH : (h + 1) * OUT_SH].T).view(np.uint32)
        for h in range(C)
    ]
    bbc = [
        np.ascontiguousarray(
            np.broadcast_to(bias[h * OUT_SH : (h + 1) * OUT_SH][None, :], (P, OUT_SH))
        )
        for h in range(C)
    ]
    maps = []
    for c in range(N_CORES):
        r, h = divmod(c, C)
        maps.append({"xT": xts[r], "wT": wts[h], "b": bbc[h]})
    return maps


def kernel(x, weight, bias):
    from concourse.bass_utils import run_bass_kernel_spmd

    x = np.ascontiguousarray(np.asarray(x, dtype=np.float32))
    weight = np.ascontiguousarray(np.asarray(weight, dtype=np.float32))
    bias = np.asarray(bias, dtype=np.float32)

    res = run_bass_kernel_spmd(
        _get_nc(), _in_maps(x, weight, bias), list(range(N_CORES))
    )

    out = np.empty((N_TOK, OUT_F), dtype=np.float32)
    for c in range(N_CORES):
        r, h = divmod(c, C)
        out[r * TOK_SH : (r + 1) * TOK_SH, h * OUT_SH : (h + 1) * OUT_SH] = res.results[
            c
        ]["y"].astype(np.float32)
    return out


def time_kernel_ns(inputs, k1=2, k2=162, reps=5):
    """HW time per kernel execution, measured as the slope between two
    hardware-loop variants (repeat=k1 vs repeat=k2) so the multi-ms axon
    dispatch cost cancels exactly."""
    import jax
    from jax.sharding import Mesh, PartitionSpec
    from jax.experimental.shard_map import shard_map
    from concourse import bass2jax
    from concourse import mybir as mb

    x = np.ascontiguousarray(np.asarray(inputs["x"], dtype=np.float32))
    weight = np.ascontiguousarray(np.asarray(inputs["weight"], dtype=np.float32))
    bias = np.asarray(inputs["bias"], dtype=np.float32)
    in_maps = _in_maps(x, weight, bias)

    def make_fn(nc):
        bass2jax.install_neuronx_cc_hook()
        partition_name = nc.partition_id_tensor.name if nc.partition_id_tensor else None
        in_names, out_names, out_avals, zero_outs = [], [], [], []
        for alloc in nc.m.functions[0].allocations:
            if not isinstance(alloc, mb.MemoryLocationSet):
                continue
            name = alloc.memorylocations[0].name
            if alloc.kind == "ExternalInput":
                if name != partition_name:
                    in_names.append(name)
            elif alloc.kind == "ExternalOutput":
                out_names.append(name)
                shape = tuple(alloc.tensor_shape)
                dtype = mb.dt.np(alloc.dtype)
                out_avals.append(jax.core.ShapedArray(shape, dtype))
                zero_outs.append(np.zeros(shape, dtype))
        n_params = len(in_names)
        all_in = in_names + out_names
        if partition_name is not None:
            all_in.append(partition_name)

        def _body(*args):
            operands = list(args)
            if partition_name is not None:
                operands.append(bass2jax.partition_id_tensor())
            return tuple(
                bass2jax._bass_exec_p.bind(
                    *operands,
                    out_avals=tuple(out_avals),
                    in_names=tuple(all_in),
                    out_names=tuple(out_names),
                    lowering_input_output_aliases=(),
                    sim_require_finite=True,
                    sim_require_nnan=True,
                    nc=nc,
                )
            )

        devices = jax.devices()[:N_CORES]
        mesh = Mesh(np.asarray(devices), ("core",))
        nin = n_params + len(out_names)
        fn = jax.jit(
            shard_map(_body, mesh=mesh, in_specs=(PartitionSpec("core"),) * nin,
                      out_specs=(PartitionSpec("core"),) * len(out_names), check_rep=False),
            keep_unused=True,
        )
        return fn, in_names[:n_params], zero_outs

    def measure(nc):
        fn, names, zero_outs = make_fn(nc)
        dev_in = [
            jax.device_put(np.concatenate([np.asarray(m[nm]) for m in in_maps], axis=0))
            for nm in names
        ]
        dev_zero = [
            jax.device_put(np.zeros((N_CORES * z.shape[0], *z.shape[1:]), z.dtype))
            for z in zero_outs
        ]
        for a in dev_in + dev_zero:
            a.block_until_ready()
        out = fn(*dev_in, *dev_zero)
        for o in out:
            o.block_until_ready()
        ts = []
        for _ in range(reps):
            t0 = time.perf_counter()
            out = fn(*dev_in, *dev_zero)
            for o in out:
                o.block_until_ready()
            ts.append(time.perf_counter() - t0)
        ts.sort()
        return ts[0]

    t1 = measure(build_nc(repeat=k1))
    t2 = measure(build_nc(repeat=k2))
    return (t2 - t1) / (k2 - k1) * 1e9


# revision 5
# speedup vs baseline: 1.1542x; 1.1542x over previous
"""BinaryLinear v14: v9 + software-pipelined repeat loop.

Same single-iteration dataflow as v9 (top-byte inputs, DVE bitwise sign to
fp8 DoubleRow tiles, staircase loads, 1024-MM fp8 DoubleRow stream). The
repeat loop is 2x-unrolled with ping-pong corner tiles so iteration i+1's
first (m,n) corner loads and signs overlap iteration i's matmul tail:
all input loads ride the ACT HWDGE ring, all y stores the SP ring, and the
bias load is hoisted out of the loop, so neither ring head-of-line blocks
the next iteration's prefetch.
"""

import os
import sys
import time

sys.path.insert(0, "/opt/trn_rl_repo")

if "jax" not in sys.modules and os.environ.get("JAX_PLATFORMS") in ("cpu",):
    del os.environ["JAX_PLATFORMS"]

import numpy as np
import ml_dtypes

import concourse.bass as bass
import concourse.mybir as mybir
import concourse.tile as tile

N_TOK = 8192
IN_F = 4096
OUT_F = 4096
R = 4
C = 2
N_CORES = 8
TOK_SH = N_TOK // R  # 2048
OUT_SH = OUT_F // C  # 2048
P = 128
KS = IN_F // P  # 32 k-subtiles
O_CHUNK = 512
M_CHUNK = 512
SB = 4  # k-slices per load/sign batch

f32 = mybir.dt.float32
bf16 = mybir.dt.bfloat16
fp8 = mybir.dt.float8e4
u32 = mybir.dt.uint32


def _split_multi_waits(nc, limit=1):
    """walrus allows one sync-wait per instruction; move extras onto
    preceding NoOps (engines are in-order, so semantics are unchanged)."""
    for f in nc.m.functions:
        for bb in f.blocks:
            new = []
            for inst in bb.instructions:
                si = inst.sync_info
                if si is not None and len(si.on_wait) > limit:
                    waits = list(si.on_wait)
                    extra, keep = waits[:-limit], waits[-limit:]
                    for j, w in enumerate(extra):
                        new.append(
                            mybir.InstNoOp(
                                name=f"{inst.name}-w{j}",
                                engine=inst.engine,
                                sync_info=mybir.SyncInfo(on_wait=[w], on_update=[]),
                            )
                        )
                    inst.sync_info = mybir.SyncInfo(
                        on_wait=keep, on_update=list(si.on_update)
                    )
                new.append(inst)
            bb.instructions = new


def build_nc(repeat=1):
    nc = bass.Bass()
    xT = nc.declare_dram_parameter("xT", [IN_F, TOK_SH // 4], u32, isOutput=False)
    wT = nc.declare_dram_parameter("wT", [IN_F, OUT_SH // 4], u32, isOutput=False)
    b = nc.declare_dram_parameter("b", [P, OUT_SH], f32, isOutput=False)
    y = nc.declare_dram_parameter("y", [TOK_SH, OUT_SH], bf16, isOutput=True)

    nparity = 2

    with tile.TileContext(nc) as tc:
        with (
            tc.tile_pool(name="const", bufs=1) as const,
            tc.tile_pool(name="stage", bufs=10) as stage,
            tc.tile_pool(name="big", bufs=1) as big,
            tc.tile_pool(name="psum", bufs=8, space="PSUM") as psum_pool,
            tc.tile_pool(name="outp", bufs=6) as out_pool,
        ):
            bias_bc = const.tile([P, OUT_SH], f32)

            # Main tiles hold m 512:2048 / n 512:2048; the first (m,n) corner
            # lives in per-parity ping-pong tiles so iteration i+1's corner
            # prefetch never write-after-read conflicts with iteration i.
            xbt = big.tile([P, KS, TOK_SH - M_CHUNK], fp8)
            wbt = big.tile([P, KS, OUT_SH - O_CHUNK], fp8)
            cxs = [
                big.tile([P, KS, M_CHUNK], fp8, name=f"cx{i}") for i in range(nparity)
            ]
            cws = [
                big.tile([P, KS, O_CHUNK], fp8, name=f"cw{i}") for i in range(nparity)
            ]

            def load_batch(src, dst, dram_c0, dst_c0, sb0, nb=SB):
                """Load k-slices [sb0, sb0+nb) of byte-cols [dram_c0,
                dram_c0+512) of src; DVE bitwise sign -> fp8 into dst."""
                cu = dram_c0 // 4
                st = stage.tile([P, SB, M_CHUNK // 4], u32, tag="st")
                nc.scalar.dma_start(
                    out=st[:, 0:nb, :],
                    in_=src[sb0 * P : (sb0 + nb) * P, cu : cu + M_CHUNK // 4].rearrange(
                        "(s p) c -> p s c", s=nb
                    ),
                )
                nc.vector.tensor_scalar(
                    dst[:, sb0 : sb0 + nb, dst_c0 : dst_c0 + M_CHUNK].bitcast(u32),
                    st[:, 0:nb, :],
                    0x80808080,
                    0x38383838,
                    mybir.AluOpType.bitwise_and,
                    mybir.AluOpType.bitwise_or,
                )

            def load_chunk(src, dst, dram_c0, dst_c0):
                for sb0 in range(0, KS, SB):
                    load_batch(src, dst, dram_c0, dst_c0, sb0)

            def load_corner(par):
                for sb0 in range(0, KS, 2):
                    load_batch(xT, cxs[par], 0, 0, sb0, nb=2)
                    load_batch(wT, cws[par], 0, 0, sb0, nb=2)

            def emit_mm(mt, oc, par):
                if mt < 4:
                    lsrc, lc0 = cxs[par], mt * P
                else:
                    lsrc, lc0 = xbt, (mt - 4) * P
                if oc == 0:
                    rsrc, rc0 = cws[par], 0
                else:
                    rsrc, rc0 = wbt, (oc - 1) * O_CHUNK
                ps = psum_pool.tile([P, O_CHUNK], f32, tag="ps")
                for kp in range(KS // 2):
                    nc.tensor.matmul(
                        ps[:],
                        lhsT=lsrc[:, 2 * kp : 2 * kp + 2, lc0 : lc0 + P],
                        rhs=rsrc[:, 2 * kp : 2 * kp + 2, rc0 : rc0 + O_CHUNK],
                        start=(kp == 0),
                        stop=(kp == KS // 2 - 1),
                        perf_mode=mybir.MatmulPerfMode.DoubleRow,
                    )
                out_sb = out_pool.tile([P, O_CHUNK], bf16, tag="out_sb")
                nc.vector.tensor_add(
                    out=out_sb[:],
                    in0=ps[:],
                    in1=bias_bc[:, oc * O_CHUNK : (oc + 1) * O_CHUNK],
                )
                row0 = mt * P
                nc.sync.dma_start(
                    out=y[row0 : row0 + P, oc * O_CHUNK : (oc + 1) * O_CHUNK],
                    in_=out_sb[:],
                )

            def body(par):
                for mt in range(4):
                    emit_mm(mt, 0, par)
                load_chunk(xT, xbt, 512, 0)
                for mt in range(4, 8):
                    emit_mm(mt, 0, par)
                load_chunk(wT, wbt, 512, 0)
                for mt in range(8):
                    emit_mm(mt, 1, par)
                load_chunk(xT, xbt, 1024, 512)
                for mt in range(8, 12):
                    emit_mm(mt, 0, par)
                    emit_mm(mt, 1, par)
                load_chunk(xT, xbt, 1536, 1024)
                for mt in range(12, 16):
                    emit_mm(mt, 0, par)
                    emit_mm(mt, 1, par)
                load_corner(1 - par)
                load_chunk(wT, wbt, 1024, 512)
                for mt in range(16):
                    emit_mm(mt, 2, par)
                load_chunk(wT, wbt, 1536, 1024)
                for mt in range(16):
                    emit_mm(mt, 3, par)

            nc.sync.dma_start(out=bias_bc[:], in_=b[:])
            load_corner(0)
            if repeat == 1:
                body(0)
            else:
                with tc.For_i(0, repeat // 2, 1):
                    body(0)
                    body(1)
                if repeat % 2:
                    body(0)

    _split_multi_waits(nc)
    return nc


_cached_nc = None


def _get_nc():
    global _cached_nc
    if _cached_nc is None:
        _cached_nc = build_nc()
    return _cached_nc


def _in_maps(x, weight, bias):
    # Top byte of each little-endian f32 = sign bit + 7 high exponent bits.
    # Pure byte movement; sign() itself runs on device (DVE bitwise).
    xv = x.view(np.uint8)[:, 3::4]  # [N_TOK, IN_F]
    wv = weight.view(np.uint8)[:, 3::4]  # [OUT_F, IN_F]
    xts = [
        np.ascontiguousarray(xv[r * TOK_SH : (r + 1) * TOK_SH].T).view(np.uint32)
        for r in range(R)
    ]
    wts = [
        np.ascontiguousarray(wv[h * OUT_SH : (h + 1) * OUT_SH].T).view(np.uint32)
        for h in range(C)
    ]
    bbc = [
        np.ascontiguousarray(
            np.broadcast_to(bias[h * OUT_SH : (h + 1) * OUT_SH][None, :], (P, OUT_SH))
        )
        for h in range(C)
    ]
    maps = []
    for c in range(N_CORES):
        r, h = divmod(c, C)
        maps.append({"xT": xts[r], "wT": wts[h], "b": bbc[h]})
    return maps


def kernel(x, weight, bias):
    from concourse.bass_utils import run_bass_kernel_spmd

    x = np.ascontiguousarray(np.asarray(x, dtype=np.float32))
    weight = np.ascontiguousarray(np.asarray(weight, dtype=np.float32))
    bias = np.asarray(bias, dtype=np.float32)

    res = run_bass_kernel_spmd(
        _get_nc(), _in_maps(x, weight, bias), list(range(N_CORES))
    )

    out = np.empty((N_TOK, OUT_F), dtype=np.float32)
    for c in range(N_CORES):
        r, h = divmod(c, C)
        out[r * TOK_SH : (r + 1) * TOK_SH, h * OUT_SH : (h + 1) * OUT_SH] = res.results[
            c
        ]["y"].astype(np.float32)
    return out


def time_kernel_ns(inputs, k1=2, k2=162, reps=5):
    """HW time per kernel execution, measured as the slope between two
    hardware-loop variants (repeat=k1 vs repeat=k2) so the multi-ms axon
    dispatch cost cancels exactly."""
    import jax
    from jax.sharding import Mesh, PartitionSpec
    from jax.experimental.shard_map import shard_map
    from concourse import bass2jax
    from concourse import mybir as mb

    x = np.ascontiguousarray(np.asarray(inputs["x"], dtype=np.float32))
    weight = np.ascontiguousarray(np.asarray(inputs["weight"], dtype=np.float32))
    bias = np.asarray(inputs["bias"], dtype=np.float32)
    in_maps = _in_maps(x, weight, bias)

    def make_fn(nc):
        bass2jax.install_neuronx_cc_hook()
        partition_name = nc.partition_id_tensor.name if nc.partition_id_tensor else None
        in_names, out_names, out_avals, zero_outs = [], [], [], []
        for alloc in nc.m.functions[0].allocations:
            if not isinstance(alloc, mb.MemoryLocationSet):
                continue
            name = alloc.memorylocations[0].name
            if alloc.kind == "ExternalInput":
                if name != partition_name:
                    in_names.append(name)
            elif alloc.kind == "ExternalOutput":
                out_names.append(name)
                shape = tuple(alloc.tensor_shape)
                dtype = mb.dt.np(alloc.dtype)
                out_avals.append(jax.core.ShapedArray(shape, dtype))
                zero_outs.append(np.zeros(shape, dtype))
        n_params = len(in_names)
        all_in = in_names + out_names
        if partition_name is not None:
            all_in.append(partition_name)

        def _body(*args):
            operands = list(args)
            if partition_name is not None:
                operands.append(bass2jax.partition_id_tensor())
            return tuple(
                bass2jax._bass_exec_p.bind(
                    *operands,
                    out_avals=tuple(out_avals),
                    in_names=tuple(all_in),
                    out_names=tuple(out_names),
                    lowering_input_output_aliases=(),
                    sim_require_finite=True,
                    sim_require_nnan=True,
                    nc=nc,
                )
            )

        devices = jax.devices()[:N_CORES]
        mesh = Mesh(np.asarray(devices), ("core",))
        nin = n_params + len(out_names)
        fn = jax.jit(
            shard_map(_body, mesh=mesh, in_specs=(PartitionSpec("core"),) * nin,
                      out_specs=(PartitionSpec("core"),) * len(out_names), check_rep=False),
            keep_unused=True,
        )
        return fn, in_names[:n_params], zero_outs

    def measure(nc):
        fn, names, zero_outs = make_fn(nc)
        dev_in = [
            jax.device_put(np.concatenate([np.asarray(m[nm]) for m in in_maps], axis=0))
            for nm in names
        ]
        dev_zero = [
            jax.device_put(np.zeros((N_CORES * z.shape[0], *z.shape[1:]), z.dtype))
            for z in zero_outs
        ]
        for a in dev_in + dev_zero:
            a.block_until_ready()
        out = fn(*dev_in, *dev_zero)
        for o in out:
            o.block_until_ready()
        ts = []
        for _ in range(reps):
            t0 = time.perf_counter()
            out = fn(*dev_in, *dev_zero)
            for o in out:
                o.block_until_ready()
            ts.append(time.perf_counter() - t0)
        ts.sort()
        return ts[0]

    t1 = measure(build_nc(repeat=k1))
    t2 = measure(build_nc(repeat=k2))
    return (t2 - t1) / (k2 - k1) * 1e9
